# revision 4
# baseline (speedup 1.0000x reference)
"""Causal self-attention (B=2, S=2048, E=1024, H=16, DH=64) on 8 trn2 cores.

Sharding: core c -> (batch b = c//4, head-group g = c%4, heads 4g..4g+3).
Data parallel over batch, tensor parallel over heads, row-sharded Wo;
partial outputs summed on host.

Per-core device kernel (bf16 matmuls, fp32 accumulation), fully pipelined:
all PSUM pools coexist (kq 1 bank + v 1 + st/out 4 + av 2 = 8 banks) so the
projection (A), attention (B) and output-projection (C) phases interleave --
A/C matmuls fill the PE gaps left by the ACT-bound exp chain in B.

  A: kqT = (x @ Wkq)^T via W-stationary matmuls on xT (+bias on DVE),
     v   =  x @ Wv  (natural layout, +ones column for row-sums, Pool evict)
  B: scores^T[sk,sq] = k q^T (2 heads row-packed in PE, K=64),
     P^T = exp(scores/8) (ACT; causal-trimmed, triu-masked diag on Pool),
     AV: saT_aug = v_aug^T @ P^T (row 64 = softmax row-sums via the ones
     column).  av bank is freed by one fast DVE copy; the normalize
     (reciprocal -> partition-broadcast -> muls on DVE+Pool) runs off-bank.
  C: out = saT^T @ Wo (row-shard), through the st PSUM ring, bf16 DMA out
     (partials summed in fp32 on host).
"""
import numpy as np
import ml_dtypes

import concourse.bass as bass
import concourse.bacc as bacc
import concourse.tile as tile
from concourse import mybir
from concourse.masks import make_upper_triangular

BF16 = mybir.dt.bfloat16
F32 = mybir.dt.float32
NP_BF16 = ml_dtypes.bfloat16

B, S, E, H, DH = 2, 2048, 1024, 16, 64
N_CORES = 8
HPC = 4          # heads per core
SCH = 4          # number of 512-wide sq chunks
SKT = 16         # number of 128-wide sk tiles
ET = 8           # number of 128-wide e tiles

Exp = mybir.ActivationFunctionType.Exp


def build_nc(reps=1):
    nc = bacc.Bacc(None, target_bir_lowering=False)

    xT = nc.dram_tensor("xT", [E, S], BF16, kind="ExternalInput")
    wkq = nc.dram_tensor("wkq", [E, 512], BF16, kind="ExternalInput")
    wv = nc.dram_tensor("wv", [E, 256], BF16, kind="ExternalInput")
    wo = nc.dram_tensor("wo", [256, E], BF16, kind="ExternalInput")
    bkq = nc.dram_tensor("bkq", [128, 4], F32, kind="ExternalInput")
    out = nc.dram_tensor("out", [S, E], BF16, kind="ExternalOutput")

    with tile.TileContext(nc) as tc:
        import contextlib
        with contextlib.ExitStack() as ctx:
            const = ctx.enter_context(tc.tile_pool(name="const", bufs=1))
            wpool = ctx.enter_context(tc.tile_pool(name="wpool", bufs=1))
            xpool = ctx.enter_context(tc.tile_pool(name="xpool", bufs=1))
            kqpool = ctx.enter_context(tc.tile_pool(name="kqpool", bufs=1))
            vpool = ctx.enter_context(tc.tile_pool(name="vpool", bufs=1))
            sapool = ctx.enter_context(tc.tile_pool(name="sapool", bufs=1))
            pt_pool = ctx.enter_context(tc.tile_pool(name="pt", bufs=6))
            avs_pool = ctx.enter_context(tc.tile_pool(name="avs", bufs=2))
            bc_pool = ctx.enter_context(tc.tile_pool(name="bc", bufs=4))
            small = ctx.enter_context(tc.tile_pool(name="small", bufs=8))
            ostage = ctx.enter_context(tc.tile_pool(name="ostage", bufs=4))
            # PSUM: kq 1 bank + v 1 + st(+out) 2x2 + av 2 = 8 banks
            kq_ps = ctx.enter_context(
                tc.tile_pool(name="kq_ps", bufs=1, space="PSUM"))
            v_ps = ctx.enter_context(
                tc.tile_pool(name="v_ps", bufs=1, space="PSUM"))
            st_ps = ctx.enter_context(
                tc.tile_pool(name="st_ps", bufs=2, space="PSUM"))
            av_ps = ctx.enter_context(
                tc.tile_pool(name="av_ps", bufs=1, space="PSUM"))

            bkq_sb = const.tile([128, 4], F32)
            nc.sync.dma_start(bkq_sb[:], bkq[:])
            triu2 = const.tile([128, 2, 128], BF16)
            make_upper_triangular(nc, triu2[:, 0, :], val=1.0, diag=True)
            make_upper_triangular(nc, triu2[:, 1, :], val=1.0, diag=True)

            # weights + chunk-0 of xT first so kq(c=0) starts ASAP; the
            # rest of xT streams in chunk-major order just ahead of use
            wkq_sb = wpool.tile([128, ET, 512], BF16)
            xT_sb = xpool.tile([128, ET, S], BF16)
            for e in range(ET):
                nc.sync.dma_start(wkq_sb[:, e, :], wkq[128 * e:128 * (e + 1), :])
                nc.sync.dma_start(xT_sb[:, e, 0:512],
                                  xT[128 * e:128 * (e + 1), 0:512])
            wv_sb = wpool.tile([128, ET, 256], BF16)
            nc.sync.dma_start(wv_sb[:], wv.rearrange("(n p) f -> p n f", p=128))
            for c in range(1, SCH):
                for e in range(ET):
                    nc.sync.dma_start(
                        xT_sb[:, e, 512 * c:512 * (c + 1)],
                        xT[128 * e:128 * (e + 1), 512 * c:512 * (c + 1)])
            wo_sb = wpool.tile([128, 2, E], BF16)
            nc.sync.dma_start(wo_sb[:], wo.rearrange("(n p) f -> p n f", p=128))

            kqT_sb = kqpool.tile([128, 4, S], BF16)      # blk: p0k,p0q,p1k,p1q
            vaug_sb = vpool.tile([128, SKT, HPC, 65], BF16)
            nc.vector.memset(vaug_sb[:, :, :, 64:65], 1.0)
            saT_sb = sapool.tile([128, 2, S], BF16)      # dim1: pair

            from collections import deque

            for _rep in range(reps):
                # ---- filler machinery: small PE work units woven between
                # attention j-iterations so the in-order PE never idles
                # (and stays out of the slow p-state) while ACT runs exp.
                filler = deque()          # (emit_fn, key) FIFO
                kq_emitted = set()        # (blk, c) emission flags
                v_emitted = set()
                slots_left = [2 * sum(4 * c + 4 for c in range(SCH))]

                def drain(n):
                    for _ in range(min(n, len(filler))):
                        fn, _key = filler.popleft()
                        fn()

                def drain_until(pred):
                    while not pred():
                        fn, _key = filler.popleft()
                        fn()

                def kq_chunk(blk, c, e0, e1, box):
                    if e0 == 0:
                        box["ps"] = kq_ps.tile([128, 512], F32, tag="kq",
                                               name=f"kqps_{blk}_{c}")
                    ps = box["ps"]
                    for e in range(e0, e1):
                        nc.tensor.matmul(
                            ps[:], wkq_sb[:, e, 128 * blk:128 * (blk + 1)],
                            xT_sb[:, e, 512 * c:512 * (c + 1)],
                            start=(e == 0), stop=(e == ET - 1))
                    if e1 == ET:
                        nc.vector.tensor_scalar_add(
                            kqT_sb[:, blk, 512 * c:512 * (c + 1)], ps[:],
                            bkq_sb[:, blk:blk + 1])
                        kq_emitted.add((blk, c))

                def push_kq(blk, c):
                    box = {}
                    filler.append(
                        (lambda: kq_chunk(blk, c, 0, 4, box), ("kq", blk, c)))
                    filler.append(
                        (lambda: kq_chunk(blk, c, 4, ET, box), ("kq", blk, c)))

                def v_tile(t):
                    ps = v_ps.tile([128, 256], F32, tag="v", name=f"vps_{t}")
                    for e in range(ET):
                        nc.tensor.matmul(
                            ps[:], xT_sb[:, e, 128 * t:128 * (t + 1)],
                            wv_sb[:, e, :],
                            start=(e == 0), stop=(e == ET - 1))
                    nc.vector.tensor_copy(
                        vaug_sb[:, t, :, 0:64],
                        ps[:].rearrange("p (h d) -> p h d", h=HPC))
                    v_emitted.add(t)

                def push_v(t):
                    filler.append((lambda: v_tile(t), ("v", t)))

                def out_tile(t):
                    ps = st_ps.tile([128, 2, 512], F32, tag="st",
                                    name=f"ops_{t}")
                    for n in range(2):
                        for pair in range(2):
                            nc.tensor.matmul(
                                ps[:, n, :],
                                saT_sb[:, pair, 128 * t:128 * (t + 1)],
                                wo_sb[:, pair, 512 * n:512 * (n + 1)],
                                start=(pair == 0), stop=(pair == 1))
                    ot = ostage.tile([128, 2, 512], BF16, tag="ot",
                                     name=f"ot_{t}")
                    nc.vector.tensor_copy(ot[:], ps[:])
                    nc.sync.dma_start(
                        out[128 * t:128 * (t + 1), :],
                        ot[:].rearrange("p n q -> p (n q)"))

                def push_out(t):
                    filler.append((lambda: out_tile(t), ("out", t)))

                def attn_segment(c, p):
                    sq0 = 512 * c
                    kblk, qblk = 2 * p, 2 * p + 1
                    nj = 4 * c + 4
                    drain_until(lambda: (
                        all((kblk, cc) in kq_emitted for cc in range(c + 1))
                        and (qblk, c) in kq_emitted
                        and all(t in v_emitted for t in range(nj))))
                    av = av_ps.tile([65, 2, 512], F32, tag="av",
                                    name=f"av_{c}_{p}")
                    pend = None       # software-pipelined AV: lags one j
                    pts = {}

                    def emit_av(j):
                        r = j - 4 * c
                        off = 128 * r if r >= 0 else 0
                        nc.tensor.matmul(av[:, 0, off:512],
                                         vaug_sb[:, j, 2 * p, :],
                                         pts[j][:, 0, off:512],
                                         start=(j == 0), stop=(j == nj - 1))
                        nc.tensor.matmul(av[:, 1, off:512],
                                         vaug_sb[:, j, 2 * p + 1, :],
                                         pts[j][:, 1, off:512],
                                         start=(j == 0), stop=(j == nj - 1))
                        del pts[j]

                    for j in range(nj):
                        r = j - 4 * c
                        diag = r >= 0
                        off = 128 * r if diag else 0
                        w = 512 - off
                        st = st_ps.tile([128, 2, 512], F32, tag="st",
                                        name=f"st_{c}_{p}_{j}")
                        nc.tensor.matmul(
                            st[:, 0, 0:w],
                            kqT_sb[0:64, kblk, 128 * j:128 * (j + 1)],
                            kqT_sb[0:64, qblk, sq0 + off:sq0 + 512],
                            start=True, stop=True, tile_position=(0, 0))
                        nc.tensor.matmul(
                            st[:, 1, 0:w],
                            kqT_sb[64:128, kblk, 128 * j:128 * (j + 1)],
                            kqT_sb[64:128, qblk, sq0 + off:sq0 + 512],
                            start=True, stop=True, tile_position=(64, 0))
                        pt = pt_pool.tile([128, 2, 512], BF16, tag="pt",
                                          name=f"pt_{c}_{p}_{j}")
                        nc.scalar.activation(pt[:, :, off:512],
                                             st[:, :, 0:w],
                                             Exp, scale=0.125)
                        if diag:
                            nc.gpsimd.tensor_mul(
                                pt[:, :, off:off + 128],
                                pt[:, :, off:off + 128], triu2[:])
                        pts[j] = pt
                        if pend is not None:
                            emit_av(pend)
                        pend = j
                        slots_left[0] -= 1
                        if filler:
                            k = -(-len(filler) // max(1, slots_left[0]))
                            drain(k)
                    emit_av(pend)
                    # fast copy frees the av bank; normalize runs off-bank
                    avs = avs_pool.tile([65, 2, 512], F32, tag="avs",
                                        name=f"avs_{c}_{p}")
                    nc.vector.tensor_copy(avs[:], av[:])
                    rc = small.tile([1, 2, 512], F32, tag="rc",
                                    name=f"rc_{c}_{p}")
                    nc.vector.reciprocal(rc[0:1, :, :], avs[64:65, :, :])
                    bc = bc_pool.tile([64, 2, 512], F32, tag="bc",
                                      name=f"bc_{c}_{p}")
                    nc.gpsimd.partition_broadcast(bc[:], rc[0:1, :, :])
                    nc.vector.tensor_mul(
                        saT_sb[0:64, p, sq0:sq0 + 512],
                        avs[0:64, 0, :], bc[:, 0, :])
                    nc.gpsimd.tensor_mul(
                        saT_sb[64:128, p, sq0:sq0 + 512],
                        avs[0:64, 1, :], bc[:, 1, :])

                # ---- ramp: just enough of A for seg(0,0), rest queued
                kq_chunk(0, 0, 0, ET, {})
                kq_chunk(1, 0, 0, ET, {})
                v_tile(0)
                push_v(1); push_v(2); push_v(3)
                push_kq(2, 0); push_kq(3, 0)
                for c in range(SCH):
                    attn_segment(c, 0)
                    attn_segment(c, 1)
                    if c + 1 < SCH:
                        push_kq(0, c + 1); push_kq(1, c + 1)
                        for t in range(4 * (c + 1), 4 * (c + 2)):
                            push_v(t)
                        push_kq(2, c + 1); push_kq(3, c + 1)
                    for t in range(4 * c, 4 * (c + 1)):
                        push_out(t)
                drain(len(filler))

    nc.compile()
    return nc


_CACHE = {}


def _build_runner():
    """Build the SPMD PJRT executable once; returns a dict with a jitted fn.

    Mirrors concourse.bass2jax.run_bass_via_pjrt but hoisted so repeated
    kernel() calls reuse the traced/compiled executable. No donation: the
    kernel DMA-writes every output element, so uninitialized output buffers
    are fine.
    """
    import jax
    from jax.sharding import Mesh, PartitionSpec
    from jax.experimental.shard_map import shard_map
    from concourse import bass2jax as b2j
    from concourse import mybir as _mybir

    if "runner" in _CACHE:
        return _CACHE["runner"]

    nc = _CACHE.get("nc")
    if nc is None:
        nc = _CACHE["nc"] = build_nc()

    b2j.install_neuronx_cc_hook()
    partition_name = (nc.partition_id_tensor.name
                      if nc.partition_id_tensor else None)

    in_names, out_names, out_avals = [], [], []
    for alloc in nc.m.functions[0].allocations:
        if not isinstance(alloc, _mybir.MemoryLocationSet):
            continue
        name = alloc.memorylocations[0].name
        if alloc.kind == "ExternalInput":
            if name != partition_name:
                in_names.append(name)
        elif alloc.kind == "ExternalOutput":
            out_names.append(name)
            out_avals.append(jax.core.ShapedArray(
                tuple(alloc.tensor_shape), _mybir.dt.np(alloc.dtype)))
    n_params = len(in_names)
    zero_out_shapes = [(a.shape, a.dtype) for a in out_avals]
    all_in_names = list(in_names) + list(out_names)
    if partition_name is not None:
        all_in_names.append(partition_name)

    def _body(*args):
        operands = list(args)
        if partition_name is not None:
            operands.append(b2j.partition_id_tensor())
        outs = b2j._bass_exec_p.bind(
            *operands,
            out_avals=tuple(out_avals),
            in_names=tuple(all_in_names),
            out_names=tuple(out_names),
            lowering_input_output_aliases=(),
            sim_require_finite=True,
            sim_require_nnan=True,
            nc=nc,
        )
        return tuple(outs)

    devices = jax.devices()[:N_CORES]
    mesh = Mesh(np.asarray(devices), ("core",))
    n_outs = len(out_names)
    in_specs = (PartitionSpec("core"),) * (n_params + n_outs)
    out_specs = (PartitionSpec("core"),) * n_outs
    fn = jax.jit(shard_map(_body, mesh=mesh, in_specs=in_specs,
                           out_specs=out_specs, check_rep=False),
                 keep_unused=True)
    runner = {
        "fn": fn,
        "in_names": in_names,
        "out_names": out_names,
        "out_avals": out_avals,
        "zero_out_shapes": zero_out_shapes,
        "mesh": mesh,
    }
    _CACHE["runner"] = runner
    return runner


def _run_spmd(in_maps):
    """Execute on 8 cores, returning list of per-core output dicts."""
    r = _build_runner()
    n_cores = N_CORES
    concat_in = [
        np.concatenate([np.asarray(in_maps[c][name]) for c in range(n_cores)],
                       axis=0)
        for name in r["in_names"]
    ]
    if "zeros" not in r:
        r["zeros"] = [np.zeros((n_cores * s[0], *s[1:]), d)
                      for s, d in r["zero_out_shapes"]]
    out_arrs = r["fn"](*concat_in, *r["zeros"])
    return [
        {name: np.asarray(out_arrs[i]).reshape(n_cores, *r["out_avals"][i].shape)[c]
         for i, name in enumerate(r["out_names"])}
        for c in range(n_cores)
    ]


def _prep_core_inputs(x, Wkqv, bkqv, Wo):
    """Host-side shard/pack. Returns (in_maps, host_bias) for 8 cores."""
    xT = [np.ascontiguousarray(x[b].T).astype(NP_BF16) for b in range(B)]
    per_g = []
    for g in range(4):
        h0 = 4 * g
        wkq = np.empty((E, 512), np.float32)
        for p in range(2):
            a, b_ = h0 + 2 * p, h0 + 2 * p + 1
            wkq[:, 256 * p:256 * p + 64] = Wkqv[a][:, 0:64]
            wkq[:, 256 * p + 64:256 * p + 128] = Wkqv[b_][:, 0:64]
            wkq[:, 256 * p + 128:256 * p + 192] = Wkqv[a][:, 64:128]
            wkq[:, 256 * p + 192:256 * p + 256] = Wkqv[b_][:, 64:128]
        wv = np.concatenate([Wkqv[h0 + h][:, 128:192] for h in range(HPC)],
                            axis=1)
        wog = Wo[256 * g:256 * (g + 1), :]
        bkq_arr = np.empty((128, 4), np.float32)
        for p in range(2):
            a, b_ = h0 + 2 * p, h0 + 2 * p + 1
            bkq_arr[0:64, 2 * p] = bkqv[a][0:64]
            bkq_arr[64:128, 2 * p] = bkqv[b_][0:64]
            bkq_arr[0:64, 2 * p + 1] = bkqv[a][64:128]
            bkq_arr[64:128, 2 * p + 1] = bkqv[b_][64:128]
        per_g.append({
            "wkq": wkq.astype(NP_BF16),
            "wv": wv.astype(NP_BF16),
            "wo": wog.astype(NP_BF16),
            "bkq": bkq_arr,
        })
    in_maps = []
    for c in range(N_CORES):
        b, g = c // 4, c % 4
        m = dict(per_g[g])
        m["xT"] = xT[b]
        in_maps.append(m)
    bv = np.concatenate([bkqv[h][128:192] for h in range(H)])
    return in_maps, bv


def kernel(x, Wkqv, bkqv, Wo, bo):
    x = np.asarray(x, np.float32)
    Wkqv = np.asarray(Wkqv, np.float32)
    bkqv = np.asarray(bkqv, np.float32)
    Wo = np.asarray(Wo, np.float32)
    bo = np.asarray(bo, np.float32)

    in_maps, bv = _prep_core_inputs(x, Wkqv, bkqv, Wo)
    results = _run_spmd(in_maps)
    partials = np.stack([results[c]["out"].astype(np.float32)
                         for c in range(N_CORES)])
    partials = partials.reshape(B, 4, S, E).sum(axis=1)
    base = bv @ Wo + bo
    return (partials + base[None, None, :]).astype(np.float32)


# revision 5
# speedup vs baseline: 11.9076x; 11.9076x over previous
"""Causal self-attention (B=2, S=2048, E=1024, H=16, DH=64) on 8 trn2 cores.

Sharding: core c -> (batch b = c//4, head-group g = c%4, heads 4g..4g+3).
Data parallel over batch, tensor parallel over heads, row-sharded Wo;
partial outputs summed on host.

Per-core device kernel (bf16 matmuls, fp32 accumulation), fully pipelined:
all PSUM pools coexist (kq 1 bank + v 1 + st/out 4 + av 2 = 8 banks) so the
projection (A), attention (B) and output-projection (C) phases interleave --
A/C matmuls fill the PE gaps left by the ACT-bound exp chain in B.

  A: kqT = (x @ Wkq)^T via W-stationary matmuls on xT (+bias on DVE),
     v   =  x @ Wv  (natural layout, +ones column for row-sums, Pool evict)
  B: scores^T[sk,sq] = k q^T (2 heads row-packed in PE, K=64),
     P^T = exp(scores/8) (ACT; causal-trimmed, triu-masked diag on Pool),
     AV: saT_aug = v_aug^T @ P^T (row 64 = softmax row-sums via the ones
     column).  av bank is freed by one fast DVE copy; the normalize
     (reciprocal -> partition-broadcast -> muls on DVE+Pool) runs off-bank.
  C: out = saT^T @ Wo (row-shard), through the st PSUM ring, bf16 DMA out
     (partials summed in fp32 on host).
"""
import numpy as np
import ml_dtypes

import concourse.bass as bass
import concourse.bacc as bacc
import concourse.tile as tile
from concourse import mybir
from concourse.masks import make_upper_triangular

BF16 = mybir.dt.bfloat16
F32 = mybir.dt.float32
NP_BF16 = ml_dtypes.bfloat16

B, S, E, H, DH = 2, 2048, 1024, 16, 64
N_CORES = 8
HPC = 4          # heads per core
SCH = 4          # number of 512-wide sq chunks
SKT = 16         # number of 128-wide sk tiles
ET = 8           # number of 128-wide e tiles

Exp = mybir.ActivationFunctionType.Exp


def build_nc(reps=1):
    nc = bacc.Bacc(None, target_bir_lowering=False)

    xT = nc.dram_tensor("xT", [E, S], BF16, kind="ExternalInput")
    wkq = nc.dram_tensor("wkq", [E, 512], BF16, kind="ExternalInput")
    wv = nc.dram_tensor("wv", [E, 256], BF16, kind="ExternalInput")
    wo = nc.dram_tensor("wo", [256, E], BF16, kind="ExternalInput")
    bkq = nc.dram_tensor("bkq", [128, 4], F32, kind="ExternalInput")
    out = nc.dram_tensor("out", [S, E], BF16, kind="ExternalOutput")

    with tile.TileContext(nc) as tc:
        import contextlib
        with contextlib.ExitStack() as ctx:
            const = ctx.enter_context(tc.tile_pool(name="const", bufs=1))
            wpool = ctx.enter_context(tc.tile_pool(name="wpool", bufs=1))
            xpool = ctx.enter_context(tc.tile_pool(name="xpool", bufs=1))
            kqpool = ctx.enter_context(tc.tile_pool(name="kqpool", bufs=1))
            vpool = ctx.enter_context(tc.tile_pool(name="vpool", bufs=1))
            sapool = ctx.enter_context(tc.tile_pool(name="sapool", bufs=1))
            pt_pool = ctx.enter_context(tc.tile_pool(name="pt", bufs=6))
            avs_pool = ctx.enter_context(tc.tile_pool(name="avs", bufs=2))
            bc_pool = ctx.enter_context(tc.tile_pool(name="bc", bufs=4))
            small = ctx.enter_context(tc.tile_pool(name="small", bufs=8))
            ostage = ctx.enter_context(tc.tile_pool(name="ostage", bufs=4))
            # PSUM: kq 1 bank + v 1 + st(+out) 2x2 + av 2 = 8 banks
            kq_ps = ctx.enter_context(
                tc.tile_pool(name="kq_ps", bufs=1, space="PSUM"))
            v_ps = ctx.enter_context(
                tc.tile_pool(name="v_ps", bufs=1, space="PSUM"))
            st_ps = ctx.enter_context(
                tc.tile_pool(name="st_ps", bufs=2, space="PSUM"))
            av_ps = ctx.enter_context(
                tc.tile_pool(name="av_ps", bufs=1, space="PSUM"))

            bkq_sb = const.tile([128, 4], F32)
            nc.sync.dma_start(bkq_sb[:], bkq[:])
            triu2 = const.tile([128, 2, 128], BF16)
            make_upper_triangular(nc, triu2[:, 0, :], val=1.0, diag=True)
            make_upper_triangular(nc, triu2[:, 1, :], val=1.0, diag=True)

            # weights + chunk-0 of xT first so kq(c=0) starts ASAP; the
            # rest of xT streams in chunk-major order just ahead of use
            wkq_sb = wpool.tile([128, ET, 512], BF16)
            xT_sb = xpool.tile([128, ET, S], BF16)
            for e in range(ET):
                nc.sync.dma_start(wkq_sb[:, e, :], wkq[128 * e:128 * (e + 1), :])
                nc.sync.dma_start(xT_sb[:, e, 0:512],
                                  xT[128 * e:128 * (e + 1), 0:512])
            wv_sb = wpool.tile([128, ET, 256], BF16)
            nc.sync.dma_start(wv_sb[:], wv.rearrange("(n p) f -> p n f", p=128))
            for c in range(1, SCH):
                for e in range(ET):
                    nc.sync.dma_start(
                        xT_sb[:, e, 512 * c:512 * (c + 1)],
                        xT[128 * e:128 * (e + 1), 512 * c:512 * (c + 1)])
            wo_sb = wpool.tile([128, 2, E], BF16)
            nc.sync.dma_start(wo_sb[:], wo.rearrange("(n p) f -> p n f", p=128))

            kqT_sb = kqpool.tile([128, 4, S], BF16)      # blk: p0k,p0q,p1k,p1q
            vaug_sb = vpool.tile([128, SKT, HPC, 65], BF16)
            nc.vector.memset(vaug_sb[:, :, :, 64:65], 1.0)
            saT_sb = sapool.tile([128, 2, S], BF16)      # dim1: pair

            from collections import deque

            for _rep in range(reps):
                # ---- filler machinery: small PE work units woven between
                # attention j-iterations so the in-order PE never idles
                # (and stays out of the slow p-state) while ACT runs exp.
                filler = deque()          # (emit_fn, key) FIFO
                kq_emitted = set()        # (blk, c) emission flags
                v_emitted = set()
                slots_left = [2 * sum(4 * c + 4 for c in range(SCH))]

                def drain(n):
                    for _ in range(min(n, len(filler))):
                        fn, _key = filler.popleft()
                        fn()

                def drain_until(pred):
                    while not pred():
                        fn, _key = filler.popleft()
                        fn()

                def kq_chunk(blk, c, e0, e1, box):
                    if e0 == 0:
                        box["ps"] = kq_ps.tile([128, 512], F32, tag="kq",
                                               name=f"kqps_{blk}_{c}")
                    ps = box["ps"]
                    for e in range(e0, e1):
                        nc.tensor.matmul(
                            ps[:], wkq_sb[:, e, 128 * blk:128 * (blk + 1)],
                            xT_sb[:, e, 512 * c:512 * (c + 1)],
                            start=(e == 0), stop=(e == ET - 1))
                    if e1 == ET:
                        nc.vector.tensor_scalar_add(
                            kqT_sb[:, blk, 512 * c:512 * (c + 1)], ps[:],
                            bkq_sb[:, blk:blk + 1])
                        kq_emitted.add((blk, c))

                def push_kq(blk, c):
                    box = {}
                    filler.append(
                        (lambda: kq_chunk(blk, c, 0, 4, box), ("kq", blk, c)))
                    filler.append(
                        (lambda: kq_chunk(blk, c, 4, ET, box), ("kq", blk, c)))

                def v_tile(t):
                    ps = v_ps.tile([128, 256], F32, tag="v", name=f"vps_{t}")
                    for e in range(ET):
                        nc.tensor.matmul(
                            ps[:], xT_sb[:, e, 128 * t:128 * (t + 1)],
                            wv_sb[:, e, :],
                            start=(e == 0), stop=(e == ET - 1))
                    nc.vector.tensor_copy(
                        vaug_sb[:, t, :, 0:64],
                        ps[:].rearrange("p (h d) -> p h d", h=HPC))
                    v_emitted.add(t)

                def push_v(t):
                    filler.append((lambda: v_tile(t), ("v", t)))

                def out_tile(t):
                    ps = st_ps.tile([128, 2, 512], F32, tag="st",
                                    name=f"ops_{t}")
                    for n in range(2):
                        for pair in range(2):
                            nc.tensor.matmul(
                                ps[:, n, :],
                                saT_sb[:, pair, 128 * t:128 * (t + 1)],
                                wo_sb[:, pair, 512 * n:512 * (n + 1)],
                                start=(pair == 0), stop=(pair == 1))
                    ot = ostage.tile([128, 2, 512], BF16, tag="ot",
                                     name=f"ot_{t}")
                    nc.vector.tensor_copy(ot[:], ps[:])
                    nc.sync.dma_start(
                        out[128 * t:128 * (t + 1), :],
                        ot[:].rearrange("p n q -> p (n q)"))

                def push_out(t):
                    filler.append((lambda: out_tile(t), ("out", t)))

                def attn_segment(c, p):
                    sq0 = 512 * c
                    kblk, qblk = 2 * p, 2 * p + 1
                    nj = 4 * c + 4
                    drain_until(lambda: (
                        all((kblk, cc) in kq_emitted for cc in range(c + 1))
                        and (qblk, c) in kq_emitted
                        and all(t in v_emitted for t in range(nj))))
                    av = av_ps.tile([65, 2, 512], F32, tag="av",
                                    name=f"av_{c}_{p}")
                    pend = None       # software-pipelined AV: lags one j
                    pts = {}

                    def emit_av(j):
                        r = j - 4 * c
                        off = 128 * r if r >= 0 else 0
                        nc.tensor.matmul(av[:, 0, off:512],
                                         vaug_sb[:, j, 2 * p, :],
                                         pts[j][:, 0, off:512],
                                         start=(j == 0), stop=(j == nj - 1))
                        nc.tensor.matmul(av[:, 1, off:512],
                                         vaug_sb[:, j, 2 * p + 1, :],
                                         pts[j][:, 1, off:512],
                                         start=(j == 0), stop=(j == nj - 1))
                        del pts[j]

                    for j in range(nj):
                        r = j - 4 * c
                        diag = r >= 0
                        off = 128 * r if diag else 0
                        w = 512 - off
                        st = st_ps.tile([128, 2, 512], F32, tag="st",
                                        name=f"st_{c}_{p}_{j}")
                        nc.tensor.matmul(
                            st[:, 0, 0:w],
                            kqT_sb[0:64, kblk, 128 * j:128 * (j + 1)],
                            kqT_sb[0:64, qblk, sq0 + off:sq0 + 512],
                            start=True, stop=True, tile_position=(0, 0))
                        nc.tensor.matmul(
                            st[:, 1, 0:w],
                            kqT_sb[64:128, kblk, 128 * j:128 * (j + 1)],
                            kqT_sb[64:128, qblk, sq0 + off:sq0 + 512],
                            start=True, stop=True, tile_position=(64, 0))
                        pt = pt_pool.tile([128, 2, 512], BF16, tag="pt",
                                          name=f"pt_{c}_{p}_{j}")
                        nc.scalar.activation(pt[:, :, off:512],
                                             st[:, :, 0:w],
                                             Exp, scale=0.125)
                        if diag:
                            nc.vector.tensor_mul(
                                pt[:, :, off:off + 128],
                                pt[:, :, off:off + 128], triu2[:])
                        pts[j] = pt
                        if pend is not None:
                            emit_av(pend)
                        pend = j
                        slots_left[0] -= 1
                        if filler:
                            k = -(-len(filler) // max(1, slots_left[0]))
                            drain(k)
                    emit_av(pend)
                    # fast copy frees the av bank; normalize runs off-bank
                    avs = avs_pool.tile([65, 2, 512], F32, tag="avs",
                                        name=f"avs_{c}_{p}")
                    nc.vector.tensor_copy(avs[:], av[:])
                    rc = small.tile([1, 2, 512], F32, tag="rc",
                                    name=f"rc_{c}_{p}")
                    nc.vector.reciprocal(rc[0:1, :, :], avs[64:65, :, :])
                    bc = bc_pool.tile([64, 2, 512], F32, tag="bc",
                                      name=f"bc_{c}_{p}")
                    nc.gpsimd.partition_broadcast(bc[:], rc[0:1, :, :])
                    nc.vector.tensor_mul(
                        saT_sb[0:64, p, sq0:sq0 + 512],
                        avs[0:64, 0, :], bc[:, 0, :])
                    nc.vector.tensor_mul(
                        saT_sb[64:128, p, sq0:sq0 + 512],
                        avs[0:64, 1, :], bc[:, 1, :])

                # ---- ramp: just enough of A for seg(0,0), rest queued
                kq_chunk(0, 0, 0, ET, {})
                kq_chunk(1, 0, 0, ET, {})
                v_tile(0)
                push_v(1); push_v(2); push_v(3)
                push_kq(2, 0); push_kq(3, 0)
                for c in range(SCH):
                    attn_segment(c, 0)
                    attn_segment(c, 1)
                    if c + 1 < SCH:
                        push_kq(0, c + 1); push_kq(1, c + 1)
                        for t in range(4 * (c + 1), 4 * (c + 2)):
                            push_v(t)
                        push_kq(2, c + 1); push_kq(3, c + 1)
                    for t in range(4 * c, 4 * (c + 1)):
                        push_out(t)
                drain(len(filler))

    nc.compile()
    return nc


_CACHE = {}


def _build_runner():
    """Build the SPMD PJRT executable once; returns a dict with a jitted fn.

    Mirrors concourse.bass2jax.run_bass_via_pjrt but hoisted so repeated
    kernel() calls reuse the traced/compiled executable. No donation: the
    kernel DMA-writes every output element, so uninitialized output buffers
    are fine.
    """
    import jax
    from jax.sharding import Mesh, PartitionSpec
    from jax.experimental.shard_map import shard_map
    from concourse import bass2jax as b2j
    from concourse import mybir as _mybir

    if "runner" in _CACHE:
        return _CACHE["runner"]

    nc = _CACHE.get("nc")
    if nc is None:
        nc = _CACHE["nc"] = build_nc()

    b2j.install_neuronx_cc_hook()
    partition_name = (nc.partition_id_tensor.name
                      if nc.partition_id_tensor else None)

    in_names, out_names, out_avals = [], [], []
    for alloc in nc.m.functions[0].allocations:
        if not isinstance(alloc, _mybir.MemoryLocationSet):
            continue
        name = alloc.memorylocations[0].name
        if alloc.kind == "ExternalInput":
            if name != partition_name:
                in_names.append(name)
        elif alloc.kind == "ExternalOutput":
            out_names.append(name)
            out_avals.append(jax.core.ShapedArray(
                tuple(alloc.tensor_shape), _mybir.dt.np(alloc.dtype)))
    n_params = len(in_names)
    zero_out_shapes = [(a.shape, a.dtype) for a in out_avals]
    all_in_names = list(in_names) + list(out_names)
    if partition_name is not None:
        all_in_names.append(partition_name)

    def _body(*args):
        operands = list(args)
        if partition_name is not None:
            operands.append(b2j.partition_id_tensor())
        outs = b2j._bass_exec_p.bind(
            *operands,
            out_avals=tuple(out_avals),
            in_names=tuple(all_in_names),
            out_names=tuple(out_names),
            lowering_input_output_aliases=(),
            sim_require_finite=True,
            sim_require_nnan=True,
            nc=nc,
        )
        return tuple(outs)

    devices = jax.devices()[:N_CORES]
    mesh = Mesh(np.asarray(devices), ("core",))
    n_outs = len(out_names)
    in_specs = (PartitionSpec("core"),) * (n_params + n_outs)
    out_specs = (PartitionSpec("core"),) * n_outs
    fn = jax.jit(shard_map(_body, mesh=mesh, in_specs=in_specs,
                           out_specs=out_specs, check_rep=False),
                 keep_unused=True)
    runner = {
        "fn": fn,
        "in_names": in_names,
        "out_names": out_names,
        "out_avals": out_avals,
        "zero_out_shapes": zero_out_shapes,
        "mesh": mesh,
    }
    _CACHE["runner"] = runner
    return runner


def _run_spmd(in_maps):
    """Execute on 8 cores, returning list of per-core output dicts."""
    r = _build_runner()
    n_cores = N_CORES
    concat_in = [
        np.concatenate([np.asarray(in_maps[c][name]) for c in range(n_cores)],
                       axis=0)
        for name in r["in_names"]
    ]
    if "zeros" not in r:
        r["zeros"] = [np.zeros((n_cores * s[0], *s[1:]), d)
                      for s, d in r["zero_out_shapes"]]
    out_arrs = r["fn"](*concat_in, *r["zeros"])
    return [
        {name: np.asarray(out_arrs[i]).reshape(n_cores, *r["out_avals"][i].shape)[c]
         for i, name in enumerate(r["out_names"])}
        for c in range(n_cores)
    ]


def _prep_core_inputs(x, Wkqv, bkqv, Wo):
    """Host-side shard/pack. Returns (in_maps, host_bias) for 8 cores."""
    xT = [np.ascontiguousarray(x[b].T).astype(NP_BF16) for b in range(B)]
    per_g = []
    for g in range(4):
        h0 = 4 * g
        wkq = np.empty((E, 512), np.float32)
        for p in range(2):
            a, b_ = h0 + 2 * p, h0 + 2 * p + 1
            wkq[:, 256 * p:256 * p + 64] = Wkqv[a][:, 0:64]
            wkq[:, 256 * p + 64:256 * p + 128] = Wkqv[b_][:, 0:64]
            wkq[:, 256 * p + 128:256 * p + 192] = Wkqv[a][:, 64:128]
            wkq[:, 256 * p + 192:256 * p + 256] = Wkqv[b_][:, 64:128]
        wv = np.concatenate([Wkqv[h0 + h][:, 128:192] for h in range(HPC)],
                            axis=1)
        wog = Wo[256 * g:256 * (g + 1), :]
        bkq_arr = np.empty((128, 4), np.float32)
        for p in range(2):
            a, b_ = h0 + 2 * p, h0 + 2 * p + 1
            bkq_arr[0:64, 2 * p] = bkqv[a][0:64]
            bkq_arr[64:128, 2 * p] = bkqv[b_][0:64]
            bkq_arr[0:64, 2 * p + 1] = bkqv[a][64:128]
            bkq_arr[64:128, 2 * p + 1] = bkqv[b_][64:128]
        per_g.append({
            "wkq": wkq.astype(NP_BF16),
            "wv": wv.astype(NP_BF16),
            "wo": wog.astype(NP_BF16),
            "bkq": bkq_arr,
        })
    in_maps = []
    for c in range(N_CORES):
        b, g = c // 4, c % 4
        m = dict(per_g[g])
        m["xT"] = xT[b]
        in_maps.append(m)
    bv = np.concatenate([bkqv[h][128:192] for h in range(H)])
    return in_maps, bv


def kernel(x, Wkqv, bkqv, Wo, bo):
    x = np.asarray(x, np.float32)
    Wkqv = np.asarray(Wkqv, np.float32)
    bkqv = np.asarray(bkqv, np.float32)
    Wo = np.asarray(Wo, np.float32)
    bo = np.asarray(bo, np.float32)

    in_maps, bv = _prep_core_inputs(x, Wkqv, bkqv, Wo)
    results = _run_spmd(in_maps)
    partials = np.stack([results[c]["out"].astype(np.float32)
                         for c in range(N_CORES)])
    partials = partials.reshape(B, 4, S, E).sum(axis=1)
    base = bv @ Wo + bo
    return (partials + base[None, None, :]).astype(np.float32)
